# revision 14
# baseline (speedup 1.0000x reference)
"""Conv2d(128->256, 3x3, pad=1) over (32,128,56,56), data-parallel across 8
NeuronCores (4 images per core), with 1D Winograd F(2,3) along y.

Per core, per image: the host precomputes the y-direction Winograd input
butterfly U_j (j=0..3, each [Cin, 28ty, 56x] bf16) and the y-transformed
weights W'_j = (G w)_j.  The device then computes, for each output-row-pair
band (7 ty rows at a time):

  M_j[co, ty, x] = sum_kx sum_ci W'_j[ci, co, kx] * U_j[ci, ty, x+kx-1]

as 12 accumulating matmuls (4 j-planes x 3 x-taps, x-padding by clipping)
into 4 PSUM banks, then the inverse transform + bias on Vector/GpSimd:

  out[2ty]   = M_0 + M_1 + M_2 + b      (t0 = M0+M1 on DVE; Y0 = (M2+b)+t0)
  out[2ty+1] = M_1 - M_2 - M_3 + b      (t1 = M1-M2 on GpSimd; Y1 = (t1+b)-M3)

This is 6 MACs/output instead of 9 (direct 3x3), cutting PE time ~1/3.
Y rows interleave (stride-2 row writes) into a bf16 output tile which is
DMA'd back; the host casts bf16 -> f32 (tolerance 2e-2 >> bf16 noise).

Latency structure follows the proven baseline: first image's U loads in
ty-quarters on the Scalar HWDGE ring while weights load split across the
Sync/GpSimd rings; zero warmup matmuls ramp the PE p-state; images 1..3
load up-front on Sync; stores alternate Sync/Scalar, final store split
small so the exit barrier waits on a small transfer.
"""

import numpy as np
import ml_dtypes

import concourse.mybir as mybir
import concourse.tile as tile
from concourse import bacc
from concourse.bass_utils import run_bass_kernel_spmd

N_CORES = 8
B, CIN, H, W = 32, 128, 56, 56
COUT, R, S = 256, 3, 3
BL = B // N_CORES          # images per core
NCOT = COUT // 128         # Cout tiles of 128
NTY = H // 2               # 28 output row-pairs
TYC = 7                    # row-pairs per PSUM band
NCH = NTY // TYC           # 4 bands per (img, cot)

MM_DT = mybir.dt.bfloat16
MM_NP = ml_dtypes.bfloat16

NWARM = 6
ADD = mybir.AluOpType.add
SUB = mybir.AluOpType.subtract

_cache = {}


def _build():
    if "nc" in _cache:
        return _cache["nc"]
    nc = bacc.Bacc("TRN2", target_bir_lowering=False, debug=False)
    f32 = mybir.dt.float32
    u_d = nc.dram_tensor("u", [BL, CIN, 4, NTY, W], MM_DT, kind="ExternalInput").ap()
    w_d = nc.dram_tensor("w", [CIN, NCOT, 4, S, 128], MM_DT, kind="ExternalInput").ap()
    b_d = nc.dram_tensor("b", [128, NCOT], f32, kind="ExternalInput").ap()
    y_d = nc.dram_tensor("y", [BL, NCOT, 128, H, W], MM_DT, kind="ExternalOutput").ap()

    with tile.TileContext(nc) as tc:
        with (
            tc.tile_pool(name="consts", bufs=1) as cpool,
            tc.tile_pool(name="uin", bufs=BL) as upool,
            tc.tile_pool(name="yout", bufs=2) as opool,
            tc.tile_pool(name="s1s", bufs=2) as s1pool,
            tc.tile_pool(name="t0s", bufs=2) as t0pool,
            tc.tile_pool(name="t1s", bufs=2) as t1pool,
            tc.tile_pool(name="c2s", bufs=2) as c2pool,
            # one single-bank PSUM pool per Winograd j-plane: per-bank
            # release lets chunk n+2's matmuls start as soon as chunk n's
            # matching plane is drained (a single 4-bank tile would wait on
            # the whole chunk's LAST reader)
            tc.tile_pool(name="ps0", bufs=2, space="PSUM") as ps0pool,
            tc.tile_pool(name="ps1", bufs=2, space="PSUM") as ps1pool,
            tc.tile_pool(name="ps2", bufs=2, space="PSUM") as ps2pool,
            tc.tile_pool(name="ps3", bufs=2, space="PSUM") as ps3pool,
        ):
            jpool = {0: ps0pool, 1: ps1pool, 2: ps2pool, 3: ps3pool}
            # --- PE prewarm: zero matmuls with no DMA dependency ---
            warm_x = cpool.tile([128, 512], MM_DT)
            nc.vector.memset(warm_x[:], 0.0)
            warm_ps = ps3pool.tile([128, 8, 64], f32, tag="ps3")
            for _ in range(NWARM):
                nc.tensor.matmul(
                    warm_ps[:], warm_x[:, 0:128], warm_x[:],
                    start=True, stop=True,
                )

            # --- constants + images ---
            # weights via GpSimd SWDGE (its queue is otherwise idle early):
            # cot0 j=1 alone first -- it is the stationary of the very first
            # matmul, ready ~3us.  Both HWDGE rings are reserved for image 0,
            # whose consumption (2.1us/chunk) outpaces a single ring.
            w_sb = cpool.tile([CIN, NCOT, 4, S, 128], MM_DT)
            b_sb = cpool.tile([128, NCOT], f32)
            nc.gpsimd.dma_start(w_sb[:, 0, 1:2], w_d[:, 0, 1:2])
            nc.gpsimd.dma_start(w_sb[:, 0, 0:1], w_d[:, 0, 0:1])
            nc.gpsimd.dma_start(w_sb[:, 0, 2:4], w_d[:, 0, 2:4])
            nc.gpsimd.dma_start(b_sb[:], b_d[:])
            nc.gpsimd.dma_start(w_sb[:, 1], w_d[:, 1])

            # image 0 in ty-band quarters (consumption order), each quarter
            # split j[0:2]/j[2:4] across the Scalar/Sync rings in parallel;
            # images 1,2 halved the same way behind image 0; image 3 SWDGE
            u_tiles = []
            u0 = upool.tile([CIN, 4, NTY, W], MM_DT, name="u_sb_0", tag="u_sb")
            for c in range(NCH):
                nc.scalar.dma_start(
                    u0[:, 0:2, TYC * c : TYC * (c + 1), :],
                    u_d[0, :, 0:2, TYC * c : TYC * (c + 1), :],
                )
                nc.sync.dma_start(
                    u0[:, 2:4, TYC * c : TYC * (c + 1), :],
                    u_d[0, :, 2:4, TYC * c : TYC * (c + 1), :],
                )
            u_tiles.append(u0)
            for img in range(1, BL):
                u_sb = upool.tile([CIN, 4, NTY, W], MM_DT, name=f"u_sb_{img}", tag="u_sb")
                if img < 3:
                    nc.scalar.dma_start(u_sb[:, 0:2], u_d[img, :, 0:2])
                    nc.sync.dma_start(u_sb[:, 2:4], u_d[img, :, 2:4])
                else:
                    nc.gpsimd.dma_start(u_sb[:], u_d[img])
                u_tiles.append(u_sb)

            nstore = 0
            for img in range(BL):
                u_sb = u_tiles[img]
                for cot in range(NCOT):
                    o_sb = opool.tile(
                        [128, H, W], MM_DT, name=f"o_sb_{img}_{cot}", tag="o_sb"
                    )
                    for ch in range(NCH):
                        ty0 = TYC * ch
                        ps = {
                            j: jpool[j].tile(
                                [128, 8, 64], f32,
                                name=f"ps{j}_{img}_{cot}_{ch}", tag=f"ps{j}",
                            )
                            for j in range(4)
                        }
                        for j in (1, 0, 2, 3):
                            # kx=1 covers the full band -> start=True first
                            for ki, kx in enumerate((1, 0, 2)):
                                ox0 = max(0, 1 - kx)
                                ox1 = min(W, W + 1 - kx)
                                nc.tensor.matmul(
                                    ps[j][:, 0:TYC, ox0:ox1],
                                    w_sb[:, cot, j, kx, :],
                                    u_sb[
                                        :, j, ty0 : ty0 + TYC,
                                        ox0 + kx - 1 : ox1 + kx - 1,
                                    ],
                                    start=(ki == 0),
                                    stop=(ki == 2),
                                )
                        # inverse transform + bias. GPSIMD cannot read PSUM;
                        # each PSUM plane is read exactly once (2x ACT, 2x
                        # DVE), the rest is SBUF-only bf16:
                        #   s1 = M1+b   (ACT, PSUM)
                        #   c2 = M2     (ACT Copy, PSUM)
                        #   t0 = M0+s1  (DVE, PSUM)
                        #   t1 = s1-c2  (Pool, SBUF)
                        #   Y0 = t0+c2  (DVE, SBUF) -> even rows
                        #   Y1 = t1-M3  (DVE, PSUM) -> odd rows
                        s1 = s1pool.tile([128, TYC, W], MM_DT, tag="s1")
                        c2 = c2pool.tile([128, TYC, W], MM_DT, tag="c2")
                        t0 = t0pool.tile([128, TYC, W], MM_DT, tag="t0")
                        t1 = t1pool.tile([128, TYC, W], MM_DT, tag="t1")
                        last = img == BL - 1 and cot == NCOT - 1 and ch == NCH - 1
                        # the very last chunk drains in two row-halves so the
                        # serial mm->s1->t0/t1->Y->store tail chain is short
                        for ra, rb in ((0, 4), (4, TYC)) if last else ((0, TYC),):
                            nc.scalar.activation(
                                s1[:, ra:rb],
                                ps[1][:, ra:rb, 0:W],
                                mybir.ActivationFunctionType.Identity,
                                bias=b_sb[:, cot : cot + 1],
                            )
                            nc.scalar.activation(
                                c2[:, ra:rb],
                                ps[2][:, ra:rb, 0:W],
                                mybir.ActivationFunctionType.Copy,
                            )
                            nc.vector.tensor_add(
                                t0[:, ra:rb], ps[0][:, ra:rb, 0:W], s1[:, ra:rb]
                            )
                            nc.gpsimd.tensor_sub(
                                t1[:, ra:rb], s1[:, ra:rb], c2[:, ra:rb]
                            )
                            nc.vector.tensor_add(
                                o_sb[:, 2 * ty0 + 2 * ra : 2 * ty0 + 2 * rb : 2, :],
                                t0[:, ra:rb],
                                c2[:, ra:rb],
                            )
                            nc.vector.tensor_sub(
                                o_sb[:, 2 * ty0 + 2 * ra + 1 : 2 * ty0 + 2 * rb : 2, :],
                                t1[:, ra:rb],
                                ps[3][:, ra:rb, 0:W],
                            )
                            # store the finished slab, alternating rings
                            r0, r1 = 2 * ty0 + 2 * ra, 2 * ty0 + 2 * rb
                            if not last:
                                eng = nc.sync if nstore % 2 == 0 else nc.scalar
                                nstore += 1
                                eng.dma_start(
                                    y_d[img, cot, :, r0:r1, :], o_sb[:, r0:r1, :]
                                )
                            elif rb != TYC:
                                nc.sync.dma_start(
                                    y_d[img, cot, :, r0:r1, :], o_sb[:, r0:r1, :]
                                )
                            else:
                                rm = r1 - 2
                                nc.sync.dma_start(
                                    y_d[img, cot, :, r0:rm, :], o_sb[:, r0:rm, :]
                                )
                                nc.scalar.dma_start(
                                    y_d[img, cot, :, rm:r1, :], o_sb[:, rm:r1, :]
                                )

    nc.compile()
    _cache["nc"] = nc
    return nc


def _in_maps(inputs, weight, bias):
    x = np.asarray(inputs, dtype=np.float32)
    # y-direction Winograd butterfly on the (row-padded) input
    xp = np.zeros((B, CIN, H + 2, W), np.float32)
    xp[:, :, 1 : H + 1] = x
    a0 = xp[:, :, 0:56:2]
    a1 = xp[:, :, 1:57:2]
    a2 = xp[:, :, 2:58:2]
    a3 = xp[:, :, 3:59:2]
    u = np.ascontiguousarray(
        np.stack([a0 - a2, a1 + a2, a2 - a1, a1 - a3], axis=2).astype(MM_NP)
    )  # [B, CIN, 4, 28, 56]

    # weights: W'_j = sum_ky G[j,ky] w[..,ky,..]; layout (ci, cot, j, kx, co)
    G = np.array(
        [[1, 0, 0], [0.5, 0.5, 0.5], [0.5, -0.5, 0.5], [0, 0, 1]], np.float32
    )
    wf = np.asarray(weight, dtype=np.float32)  # (co, ci, ky, kx)
    wj = np.einsum("jk,oiky->oijy", G, wf)     # (co, ci, j, kx)
    wt = (
        wj.reshape(NCOT, 128, CIN, 4, S)
        .transpose(2, 0, 3, 4, 1)               # (ci, cot, j, kx, co)
        .astype(MM_NP)
    )
    w = np.ascontiguousarray(wt)
    b = np.ascontiguousarray(
        np.asarray(bias).astype(np.float32).reshape(NCOT, 128).T
    )
    return [
        {"u": np.ascontiguousarray(u[c * BL : (c + 1) * BL]), "w": w, "b": b}
        for c in range(N_CORES)
    ]


def kernel(inputs, weight, bias):
    nc = _build()
    in_maps = _in_maps(inputs, weight, bias)
    res = run_bass_kernel_spmd(nc, in_maps, core_ids=list(range(N_CORES)))
    out = np.concatenate(
        [res.results[c]["y"] for c in range(N_CORES)], axis=0
    )  # [B, NCOT, 128, H, W] bf16
    return np.ascontiguousarray(out.reshape(B, COUT, H, W).astype(np.float32))
